# revision 1
# baseline (speedup 1.0000x reference)
"""Kernel attention (linear attention w/ elu+1 feature map) on 8 trn2 NeuronCores.

Problem: B=8, H=8, N=1024, D=64.
  phi(x) = elu(x) + 1
  S  = phi(Q) @ phi(K)^T          [B,H,N,N]
  P  = S @ V                      [B,H,N,N]  (dv == N)
  out = P / S                     elementwise

Sharding: batch b -> core b (8 heads per core, fully independent).

Per-core dataflow (per head):
  - load Q,K [1024,64], compute phi on-chip (fp32)
  - PE-transpose (2 heads packed per 128x128 transpose) -> phiQT/phiKT [64,1024]
  - ST[m,n] = phiK @ phiQ^T via f32r matmuls (lhsT=phiKT chunk), PSUM->SBUF cast to bf16
  - V loaded fp32, cast to bf16
  - per n-chunk (128 rows):
      S chunk via f32r matmul (lhsT=phiQT chunk)
      recipS = exp(-ln(S)) on ACT (ln/exp share one LUT table set)
      P chunk = sum_m ST^T-chunk @ V-chunk (bf16 matmuls, fp32 PSUM accum)
      out = P * recipS on DVE, DMA out
"""

import numpy as np
from contextlib import ExitStack

import concourse.bass as bass
import concourse.tile as tile
import concourse.mybir as mybir
from concourse import bacc
from concourse.bass_utils import run_bass_kernel_spmd
from concourse.masks import make_identity

P = 128
N_CORES = 8
HPC = 8          # heads per core (= H; batch is the sharded dim)
N = 1024
D = 64
NT = N // P      # 8
F32 = mybir.dt.float32
F32R = mybir.dt.float32r
BF16 = mybir.dt.bfloat16
AF = mybir.ActivationFunctionType
ALU = mybir.AluOpType

_cache = {}


def _patch_act_tables():
    """Force Exp and Ln to resolve to the single table set containing both
    (natural_log_exp_and_others), so the ACT LUT is loaded once instead of
    thrashing ~2.7us per Ln<->Exp alternation. Keys/order preserved, so
    act_func_set_id indices stay valid."""
    if _cache.get("tables_patched"):
        return
    orig = bacc.get_activation_tables

    def patched(arch):
        tabs = dict(orig(arch))
        both = [k for k, v in tabs.items() if AF.Exp in v and AF.Ln in v]
        if both:
            keep = both[0]
            tabs = {
                k: (v if k == keep else (set(v) - {AF.Exp, AF.Ln}))
                for k, v in tabs.items()
            }
        return tabs

    bacc.get_activation_tables = patched
    _cache["tables_patched"] = True


def _build():
    _patch_act_tables()
    nc = bacc.Bacc("TRN2", target_bir_lowering=False, debug=False, num_devices=N_CORES)
    Q = nc.dram_tensor("q", [HPC, N, D], F32, kind="ExternalInput").ap()
    K = nc.dram_tensor("k", [HPC, N, D], F32, kind="ExternalInput").ap()
    V = nc.dram_tensor("v", [HPC, N, N], F32, kind="ExternalInput").ap()
    O = nc.dram_tensor("o", [HPC, N, N], F32, kind="ExternalOutput").ap()

    Qr = Q.rearrange("h (t p) d -> h p t d", p=P)   # [8, 128, 8, 64]
    Kr = K.rearrange("h (t p) d -> h p t d", p=P)
    Vr = V.rearrange("h (m p) v -> h p m v", p=P)   # [8, 128, 8, 1024]

    with tile.TileContext(nc) as tc, ExitStack() as ctx:
        const = ctx.enter_context(tc.tile_pool(name="const", bufs=1))
        prep = ctx.enter_context(tc.tile_pool(name="prep", bufs=2))
        qkt = ctx.enter_context(tc.tile_pool(name="qkt", bufs=2))
        stp = ctx.enter_context(tc.tile_pool(name="stp", bufs=2))
        vp = ctx.enter_context(tc.tile_pool(name="vp", bufs=2))
        vstage = ctx.enter_context(tc.tile_pool(name="vstage", bufs=4))
        outp = ctx.enter_context(tc.tile_pool(name="outp", bufs=3))
        recp = ctx.enter_context(tc.tile_pool(name="recp", bufs=2))
        tps = ctx.enter_context(tc.tile_pool(name="tpsum", bufs=2, space="PSUM"))
        sps = ctx.enter_context(tc.tile_pool(name="spsum", bufs=2, space="PSUM"))
        pps = ctx.enter_context(tc.tile_pool(name="ppsum", bufs=2, space="PSUM"))

        ident = const.tile([P, P], F32)
        make_identity(nc, ident)

        for pair in range(HPC // 2):
            h0 = 2 * pair
            h1 = 2 * pair + 1
            # ---- phase A: load Q,K both heads, phi, transpose (2 heads packed)
            qT = [qkt.tile([D, N], F32R, tag=f"qT{i}", name=f"qT{i}") for i in range(2)]
            kT = [qkt.tile([D, N], F32R, tag=f"kT{i}", name=f"kT{i}") for i in range(2)]
            for raw_tag, src, dstT in (("qraw", Qr, qT), ("kraw", Kr, kT)):
                raw = prep.tile([P, NT, 2 * D], F32, tag=raw_tag, name=raw_tag)
                nc.sync.dma_start(raw[:, :, 0:D], src[h0])
                nc.sync.dma_start(raw[:, :, D:2 * D], src[h1])
                flat = raw.rearrange("p t d -> p (t d)")
                tmp = prep.tile([P, NT * 2 * D], F32, tag="tmp")
                # phi(x) = max(x+1, exp(min(x, 0)))
                nc.vector.tensor_scalar_min(tmp[:], flat, 0.0)
                nc.scalar.activation(tmp[:], tmp[:], AF.Exp)
                nc.vector.scalar_tensor_tensor(
                    flat, flat, 1.0, tmp[:], ALU.add, ALU.max
                )
                for t in range(NT):
                    ps = tps.tile([P, P], F32)
                    nc.tensor.transpose(ps[:], raw[:, t, :], ident[:])
                    nc.scalar.copy(dstT[0][:, t * P:(t + 1) * P], ps[0:D, :])
                    nc.vector.tensor_copy(dstT[1][:, t * P:(t + 1) * P], ps[D:2 * D, :])

            for hi, h in enumerate((h0, h1)):
                qTh = qT[hi]
                kTh = kT[hi]
                # ---- phase B: ST = phiK @ phiQ^T (m on partitions), cast bf16
                st = stp.tile([P, NT, N], BF16, tag="st")
                for m in range(NT):
                    s_ps = sps.tile([P, N], F32, tag="sps")
                    for half in range(2):
                        nc.tensor.matmul(
                            s_ps[:, half * 512:(half + 1) * 512],
                            kTh[:, m * P:(m + 1) * P],
                            qTh[:, half * 512:(half + 1) * 512],
                            start=True, stop=True,
                        )
                    nc.vector.tensor_copy(st[:, m, :], s_ps[:])
                # ---- V load with fp32->bf16 cast during DMA (SWDGE)
                vt = vp.tile([P, NT, N], BF16, tag="vt")
                for m in range(NT):
                    nc.gpsimd.dma_start(vt[:, m, :], Vr[h, :, m, :])
                # ---- phase C: per n-chunk
                for n in range(NT):
                    s_ps = sps.tile([P, N], F32, tag="sps")
                    for half in range(2):
                        nc.tensor.matmul(
                            s_ps[:, half * 512:(half + 1) * 512],
                            qTh[:, n * P:(n + 1) * P],
                            kTh[:, half * 512:(half + 1) * 512],
                            start=True, stop=True,
                        )
                    lnt = recp.tile([P, N], F32, tag="ln")
                    rec = recp.tile([P, N], F32, tag="rec")
                    nc.scalar.activation(lnt[:], s_ps[:], AF.Ln)
                    nc.scalar.activation(rec[:], lnt[:], AF.Exp, scale=-1.0)
                    outt = outp.tile([P, N], F32, tag="outt")
                    for v in range(2):
                        p_ps = pps.tile([P, 512], F32, tag="pp")
                        for m in range(NT):
                            nc.tensor.matmul(
                                p_ps[:],
                                st[:, m, n * P:(n + 1) * P],
                                vt[:, m, v * 512:(v + 1) * 512],
                                start=(m == 0), stop=(m == NT - 1),
                            )
                        nc.vector.tensor_mul(
                            outt[:, v * 512:(v + 1) * 512],
                            p_ps[:],
                            rec[:, v * 512:(v + 1) * 512],
                        )
                    nc.sync.dma_start(O[h, n * P:(n + 1) * P, :], outt[:])
    nc.compile()
    return nc


def _get_nc():
    if "nc" not in _cache:
        _cache["nc"] = _build()
    return _cache["nc"]


def kernel(Q, K, V, _want_trace=False):
    """Takes full inputs Q,K [8,8,1024,64], V [8,8,1024,1024]; returns [8,8,1024,1024]."""
    nc = _get_nc()
    Q = np.ascontiguousarray(np.asarray(Q), dtype=np.float32)
    K = np.ascontiguousarray(np.asarray(K), dtype=np.float32)
    V = np.ascontiguousarray(np.asarray(V), dtype=np.float32)
    in_maps = [
        {"q": Q[b], "k": K[b], "v": V[b]} for b in range(N_CORES)
    ]
    res = run_bass_kernel_spmd(
        nc, in_maps, core_ids=list(range(N_CORES)), trace=_want_trace
    )
    out = np.stack([res.results[b]["o"] for b in range(N_CORES)], axis=0)
    if _want_trace:
        _cache["last_result"] = res
    return out



# revision 9
# speedup vs baseline: 1.4124x; 1.4124x over previous
"""Kernel attention (linear attention w/ elu+1 feature map) on 8 trn2 NeuronCores.

Problem: B=8, H=8, N=1024, D=64.
  phi(x) = elu(x) + 1
  S   = phi(Q) @ phi(K)^T          [B,H,N,N]
  out = (S @ V) / S                elementwise divide (dv == N)

Key algebraic rewrite: S has rank D=64, so the numerator is computed as
  numerator = phi(Q) @ (phi(K)^T @ V)
which is ~8x less PE work than materializing S @ V densely.  The full S is
still materialized (in PSUM, chunk by chunk) for the elementwise division,
computed as exp(-ln(S)) on the ACT engine (S > 0 always).

Sharding: batch b -> core b (8 heads per core, fully independent).

Row-block layout: partition p holds rows 8p..8p+7 (j = 0..7), so every
HBM<->SBUF transfer is >=2KB contiguous per partition (full DMA rate):
  - V[h] loads as one 4MB DMA (32KB contiguous per partition)
  - Q[h]/K[h] load as one 256KB DMA each (2KB contiguous per partition)
  - outputs store as [128, 2048] tiles (8KB contiguous per partition)
Per-core dataflow (per head):
  - phiK row-blocks are directly the lhsT for KtV = phi(K)^T @ V (contract
    over n on partitions, accumulated over the 8 j-slots)
  - phiQ/phiK are PE-transposed into qT/kT pair tiles [64(d), j, p] (even
    head on partitions 0:64, odd head on 64:128)
  - per j-chunk (rows n = 8p+j): S = qT-slice^T @ kT (f32r), rec =
    exp(-ln(S)) on ACT, num = qT-slice^T @ KtV (f32r), out = num * rec on
    DVE (with a free-dim permuted view pairing S's (j',p') column order
    with num's natural v order), DMA out per j-pair.
All matmuls run in f32r: full PE rate at free-size >= 256, fp32 storage.
"""

import numpy as np
from contextlib import ExitStack

import concourse.bass as bass
import concourse.tile as tile
import concourse.mybir as mybir
from concourse import bacc
from concourse.bass_utils import run_bass_kernel_spmd
from concourse.masks import make_identity

P = 128
N_CORES = 8
H = 8            # heads per core (batch is the sharded dim)
N = 1024
D = 64
J = N // P       # 8 rows per partition (row-block layout)
F32 = mybir.dt.float32
F32R = mybir.dt.float32r
BF16 = mybir.dt.bfloat16
AF = mybir.ActivationFunctionType
ALU = mybir.AluOpType

_cache = {}


def _patch_act_tables():
    """Force Exp and Ln to resolve to the single table set containing both
    (natural_log_exp_and_others), so the ACT LUT is loaded once instead of
    thrashing ~1.3-2.7us per Ln<->Exp alternation.  Keys/order preserved, so
    act_func_set_id indices stay valid."""
    if _cache.get("tables_patched"):
        return
    orig = bacc.get_activation_tables

    def patched(arch):
        tabs = dict(orig(arch))
        both = [k for k, v in tabs.items() if AF.Exp in v and AF.Ln in v]
        if both:
            keep = both[0]
            tabs = {
                k: (v if k == keep else (set(v) - {AF.Exp, AF.Ln}))
                for k, v in tabs.items()
            }
        return tabs

    bacc.get_activation_tables = patched
    _cache["tables_patched"] = True


def _build():
    _patch_act_tables()
    nc = bacc.Bacc("TRN2", target_bir_lowering=False, debug=False, num_devices=N_CORES)
    Q = nc.dram_tensor("q", [H, N, D], F32, kind="ExternalInput").ap()
    K = nc.dram_tensor("k", [H, N, D], F32, kind="ExternalInput").ap()
    V = nc.dram_tensor("v", [H, N, N], F32, kind="ExternalInput").ap()
    O = nc.dram_tensor("o", [H, N, N], F32, kind="ExternalOutput").ap()

    # Row-block views: partition p <- rows 8p..8p+7.
    Qr = Q.rearrange("h (p j) d -> h p (j d)", p=P)                  # [8, 128, 512]
    Kr = K.rearrange("h (p j) d -> h p (j d)", p=P)
    Vr = V.rearrange("h (p j) v -> h p (j v)", p=P)                  # [8, 128, 8192]
    Orr = O.rearrange("h (p jj jt) v -> h p jj (jt v)", p=P, jt=2)   # [8, 128, 4, 2048]

    with tile.TileContext(nc) as tc, ExitStack() as ctx:
        const = ctx.enter_context(tc.tile_pool(name="const", bufs=1))
        rawq = ctx.enter_context(tc.tile_pool(name="rawq", bufs=2))
        rawk = ctx.enter_context(tc.tile_pool(name="rawk", bufs=2))
        # kphi (bf16 phi(K), lhsT of KtV) persists until its head's KtV
        kphp = ctx.enter_context(tc.tile_pool(name="kphp", bufs=8))
        tmpp = ctx.enter_context(tc.tile_pool(name="tmpp", bufs=2))
        qkt = ctx.enter_context(tc.tile_pool(name="qkt", bufs=1))
        ktvp = ctx.enter_context(tc.tile_pool(name="ktvp", bufs=2))
        vp = ctx.enter_context(tc.tile_pool(name="vp", bufs=2))
        recp = ctx.enter_context(tc.tile_pool(name="recp", bufs=2))
        outp = ctx.enter_context(tc.tile_pool(name="outp", bufs=2))
        # PSUM: "big" ring (4 banks) shared by transposes / KtV / S chunks,
        # "nps" ring (4 banks) for numerator chunks.
        bigp = ctx.enter_context(tc.tile_pool(name="bigp", bufs=2, space="PSUM"))
        nps = ctx.enter_context(tc.tile_pool(name="nps", bufs=2, space="PSUM"))

        ident = const.tile([P, P], F32)
        make_identity(nc, ident)

        vt = [None] * H

        def load_v(h):
            # SWDGE load with fp32->bf16 cast (halves SBUF-side DMA bytes)
            v_t = vp.tile([P, J * N], BF16, tag="vt", name=f"vt{h}")
            nc.gpsimd.dma_start(v_t, Vr[h])
            vt[h] = v_t

        # Pair tiles: even head's 64 d-rows on partitions 0:64, odd on 64:128.
        qT = [None] * (H // 2)
        kT = [None] * (H // 2)
        kphis = [None] * H

        def prep_head(h):
            """Load Q/K for head h, apply phi, build its half of qT/kT."""
            pr, hh = divmod(h, 2)
            base = hh * D
            qr_t = rawq.tile([P, J * D], F32, tag="qraw", name=f"qraw{h}")
            kr_t = rawk.tile([P, J * D], F32, tag="kraw", name=f"kraw{h}")
            nc.sync.dma_start(qr_t, Qr[h])
            nc.sync.dma_start(kr_t, Kr[h])
            if hh == 0:
                qT[pr] = qkt.tile([P, J, P], F32R, tag=f"qT{pr}", name=f"qT{pr}")
                kT[pr] = qkt.tile([P, J, P], F32R, tag=f"kT{pr}", name=f"kT{pr}")
            for ri, (raw_t, dstT) in enumerate(((qr_t, qT[pr]), (kr_t, kT[pr]))):
                tmp = tmpp.tile([P, J * D], F32, tag="tmp")
                # phi(x) = elu(x) + 1 = max(x + 1, exp(min(x, 0)))
                nc.vector.tensor_scalar_min(tmp[:], raw_t[:], 0.0)
                nc.scalar.activation(tmp[:], tmp[:], AF.Exp)
                nc.vector.scalar_tensor_tensor(
                    raw_t[:], raw_t[:], 1.0, tmp[:], ALU.add, ALU.max
                )
                # transpose 2 j-slots at a time: [128, (2t,2t+1)x64] ->
                # psum rows 0:64 = j=2t d-rows, 64:128 = j=2t+1 d-rows
                for t in range(J // 2):
                    ps = bigp.tile([P, P], F32, tag="big", name="tps")
                    nc.tensor.transpose(
                        ps[:], raw_t[:, 2 * t * D:(2 * t + 2) * D], ident[:]
                    )
                    if (t + ri) % 2 == 0:
                        nc.scalar.copy(dstT[base:base + D, 2 * t, :], ps[0:D, :])
                        nc.vector.tensor_copy(
                            dstT[base:base + D, 2 * t + 1, :], ps[D:2 * D, :]
                        )
                    else:
                        nc.vector.tensor_copy(dstT[base:base + D, 2 * t, :], ps[0:D, :])
                        nc.scalar.copy(dstT[base:base + D, 2 * t + 1, :], ps[D:2 * D, :])
            # bf16 copy of phi(K) row-blocks: lhsT for the bf16 KtV matmuls
            kphi = kphp.tile([P, J * D], BF16, tag="kphi", name=f"kphi{h}")
            nc.vector.tensor_copy(kphi[:], kr_t[:])
            kphis[h] = kphi

        # Emission order chooses DMA order (all DMAs issue in-order on SP):
        # head0/1 QK, V0, heads 2/3 QK, V1, rest of QK, then the head loop
        # interleaves V[h+2] with output stores.
        prep_head(0)
        prep_head(1)
        load_v(0)
        prep_head(2)
        prep_head(3)
        load_v(1)
        prep_head(4)
        prep_head(5)
        prep_head(6)
        prep_head(7)

        for h in range(H):
            pr, hh = divmod(h, 2)
            base = hh * D  # partition base for this head's d-rows
            kphi = kphis[h]
            v_t = vt[h]

            # KtV[d, v] = sum_n phiK[n, d] V[n, v], accumulated over j-slots
            kv_ps = bigp.tile([P, N], F32, tag="big", name=f"kv{h}")
            for half in range(2):
                for j in range(J):
                    nc.tensor.matmul(
                        kv_ps[base:base + D, half * 512:(half + 1) * 512],
                        kphi[:, j * D:(j + 1) * D],
                        v_t[:, j * N + half * 512:j * N + (half + 1) * 512],
                        start=(j == 0), stop=(j == J - 1),
                    )
            ktv = ktvp.tile([P, N], F32R, tag="ktv", name=f"ktv{h}")
            nc.vector.tensor_copy(ktv[base:base + D, :], kv_ps[base:base + D, :])

            if h + 2 < H:
                load_v(h + 2)

            qTf = qT[pr]
            kTf = kT[pr].rearrange("p j q -> p (j q)")
            out_t = None
            for j in range(J):
                # S chunk: rows n = 8p+j, columns m in (j', p') order
                s_ps = bigp.tile([P, N], F32, tag="big", name=f"sps{h}_{j}")
                for half in range(2):
                    nc.tensor.matmul(
                        s_ps[:, half * 512:(half + 1) * 512],
                        qTf[base:base + D, j, :],
                        kTf[base:base + D, half * 512:(half + 1) * 512],
                        start=True, stop=True,
                    )
                lnt = recp.tile([P, N], F32, tag="ln", bufs=1)
                rec = recp.tile([P, N], F32, tag="rec")
                nc.scalar.activation(lnt[:], s_ps[:], AF.Ln)
                nc.scalar.activation(rec[:], lnt[:], AF.Exp, scale=-1.0)
                # numerator chunk: natural v order
                n_ps = nps.tile([P, N], F32, tag="nps", name=f"nps{h}_{j}")
                for half in range(2):
                    nc.tensor.matmul(
                        n_ps[:, half * 512:(half + 1) * 512],
                        qTf[base:base + D, j, :],
                        ktv[base:base + D, half * 512:(half + 1) * 512],
                        start=True, stop=True,
                    )
                if j % 2 == 0:
                    out_t = outp.tile([P, 2, N], F32, tag="out", name=f"out{h}_{j // 2}")
                # out[n, v] = num[n, v] * rec[n, m=v]; v = 8*pp + jj maps to
                # rec column (j'=jj, p'=pp) i.e. free index jj*128 + pp
                nc.vector.tensor_mul(
                    out_t[:, j % 2, :].rearrange("p (pp jj) -> p pp jj", jj=J),
                    n_ps.rearrange("p (pp jj) -> p pp jj", jj=J),
                    rec.rearrange("p (jp pp) -> p pp jp", pp=P),
                )
                if j % 2 == 1:
                    nc.sync.dma_start(
                        Orr[h, :, j // 2, :],
                        out_t.rearrange("p a b -> p (a b)"),
                    )
    nc.compile()
    return nc


def _get_nc():
    if "nc" not in _cache:
        _cache["nc"] = _build()
    return _cache["nc"]


def kernel(Q, K, V, _want_trace=False):
    """Takes full inputs Q,K [8,8,1024,64], V [8,8,1024,1024]; returns [8,8,1024,1024]."""
    nc = _get_nc()
    Q = np.ascontiguousarray(np.asarray(Q), dtype=np.float32)
    K = np.ascontiguousarray(np.asarray(K), dtype=np.float32)
    V = np.ascontiguousarray(np.asarray(V), dtype=np.float32)
    in_maps = [
        {"q": Q[b], "k": K[b], "v": V[b]} for b in range(N_CORES)
    ]
    try:
        res = run_bass_kernel_spmd(
            nc, in_maps, core_ids=list(range(N_CORES)), trace=_want_trace
        )
    except ModuleNotFoundError:
        # NTFF profiling hook unavailable in this container; rerun untraced.
        res = run_bass_kernel_spmd(
            nc, in_maps, core_ids=list(range(N_CORES)), trace=False
        )
    out = np.stack([res.results[b]["o"] for b in range(N_CORES)], axis=0)
    if _want_trace:
        _cache["last_result"] = res
    return out


# revision 12
# speedup vs baseline: 1.4143x; 1.0013x over previous
"""Kernel attention (linear attention w/ elu+1 feature map) on 8 trn2 NeuronCores.

Problem: B=8, H=8, N=1024, D=64.
  phi(x) = elu(x) + 1
  S   = phi(Q) @ phi(K)^T          [B,H,N,N]
  out = (S @ V) / S                elementwise divide (dv == N)

Key algebraic rewrite: S has rank D=64, so the numerator is computed as
  numerator = phi(Q) @ (phi(K)^T @ V)
which is ~8x less PE work than materializing S @ V densely.  The full S is
still materialized (in PSUM, chunk by chunk) for the elementwise division,
computed as exp(-ln(S)) on the ACT engine (S > 0 always).

Sharding: batch b -> core b (8 heads per core, fully independent).

Row-block layout: partition p holds rows 8p..8p+7 (j = 0..7), so every
HBM<->SBUF transfer is >=2KB contiguous per partition (full DMA rate):
  - V[h] loads as one 4MB DMA (32KB contiguous per partition)
  - Q[h]/K[h] load as one 256KB DMA each (2KB contiguous per partition)
  - outputs store as [128, 2048] tiles (8KB contiguous per partition)
Per-core dataflow (per head):
  - phiK row-blocks are directly the lhsT for KtV = phi(K)^T @ V (contract
    over n on partitions, accumulated over the 8 j-slots)
  - phiQ/phiK are PE-transposed into qT/kT pair tiles [64(d), j, p] (even
    head on partitions 0:64, odd head on 64:128)
  - per j-chunk (rows n = 8p+j): S = qT-slice^T @ kT (f32r), rec =
    exp(-ln(S)) on ACT, num = qT-slice^T @ KtV (f32r), out = num * rec on
    DVE (with a free-dim permuted view pairing S's (j',p') column order
    with num's natural v order), DMA out per j-pair.
All matmuls run in f32r: full PE rate at free-size >= 256, fp32 storage.
"""

import numpy as np
from contextlib import ExitStack

import concourse.bass as bass
import concourse.tile as tile
import concourse.mybir as mybir
from concourse import bacc
from concourse.bass_utils import run_bass_kernel_spmd
from concourse.masks import make_identity

P = 128
N_CORES = 8
H = 8            # heads per core (batch is the sharded dim)
N = 1024
D = 64
J = N // P       # 8 rows per partition (row-block layout)
F32 = mybir.dt.float32
F32R = mybir.dt.float32r
BF16 = mybir.dt.bfloat16
AF = mybir.ActivationFunctionType
ALU = mybir.AluOpType

_cache = {}


def _patch_act_tables():
    """Force Exp and Ln to resolve to the single table set containing both
    (natural_log_exp_and_others), so the ACT LUT is loaded once instead of
    thrashing ~1.3-2.7us per Ln<->Exp alternation.  Keys/order preserved, so
    act_func_set_id indices stay valid."""
    if _cache.get("tables_patched"):
        return
    orig = bacc.get_activation_tables

    def patched(arch):
        tabs = dict(orig(arch))
        both = [k for k, v in tabs.items() if AF.Exp in v and AF.Ln in v]
        if both:
            keep = both[0]
            tabs = {
                k: (v if k == keep else (set(v) - {AF.Exp, AF.Ln}))
                for k, v in tabs.items()
            }
        return tabs

    bacc.get_activation_tables = patched
    _cache["tables_patched"] = True


def _build():
    _patch_act_tables()
    nc = bacc.Bacc("TRN2", target_bir_lowering=False, debug=False, num_devices=N_CORES)
    Q = nc.dram_tensor("q", [H, N, D], F32, kind="ExternalInput").ap()
    K = nc.dram_tensor("k", [H, N, D], F32, kind="ExternalInput").ap()
    V = nc.dram_tensor("v", [H, N, N], F32, kind="ExternalInput").ap()
    O = nc.dram_tensor("o", [H, N, N], F32, kind="ExternalOutput").ap()

    # Row-block views: partition p <- rows 8p..8p+7.
    Qr = Q.rearrange("h (p j) d -> h p (j d)", p=P)                  # [8, 128, 512]
    Kr = K.rearrange("h (p j) d -> h p (j d)", p=P)
    Vr = V.rearrange("h (p j) v -> h p (j v)", p=P)                  # [8, 128, 8192]
    Orr = O.rearrange("h (p jj jt) v -> h p jj (jt v)", p=P, jt=2)   # [8, 128, 4, 2048]

    with tile.TileContext(nc) as tc, ExitStack() as ctx:
        const = ctx.enter_context(tc.tile_pool(name="const", bufs=1))
        rawq = ctx.enter_context(tc.tile_pool(name="rawq", bufs=2))
        rawk = ctx.enter_context(tc.tile_pool(name="rawk", bufs=2))
        # kphi (bf16 phi(K), lhsT of KtV) persists until its head's KtV
        kphp = ctx.enter_context(tc.tile_pool(name="kphp", bufs=8))
        tmpp = ctx.enter_context(tc.tile_pool(name="tmpp", bufs=2))
        qkt = ctx.enter_context(tc.tile_pool(name="qkt", bufs=1))
        ktvp = ctx.enter_context(tc.tile_pool(name="ktvp", bufs=2))
        vp = ctx.enter_context(tc.tile_pool(name="vp", bufs=2))
        recp = ctx.enter_context(tc.tile_pool(name="recp", bufs=2))
        outp = ctx.enter_context(tc.tile_pool(name="outp", bufs=2))
        # PSUM: "big" ring (4 banks) shared by transposes / KtV / S chunks,
        # "nps" ring (4 banks) for numerator chunks.
        bigp = ctx.enter_context(tc.tile_pool(name="bigp", bufs=2, space="PSUM"))
        nps = ctx.enter_context(tc.tile_pool(name="nps", bufs=2, space="PSUM"))

        ident = const.tile([P, P], F32)
        make_identity(nc, ident)

        vt = [None] * H

        def load_v(h):
            # SWDGE load with fp32->bf16 cast (halves SBUF-side DMA bytes)
            v_t = vp.tile([P, J * N], BF16, tag="vt", name=f"vt{h}")
            nc.gpsimd.dma_start(v_t, Vr[h])
            vt[h] = v_t

        # Pair tiles: even head's 64 d-rows on partitions 0:64, odd on 64:128.
        qT = [None] * (H // 2)
        kT = [None] * (H // 2)
        kphis = [None] * H

        def prep_head(h):
            """Load Q/K for head h, apply phi, build its half of qT/kT."""
            pr, hh = divmod(h, 2)
            base = hh * D
            qr_t = rawq.tile([P, J * D], F32, tag="qraw", name=f"qraw{h}")
            kr_t = rawk.tile([P, J * D], F32, tag="kraw", name=f"kraw{h}")
            nc.sync.dma_start(qr_t, Qr[h])
            nc.sync.dma_start(kr_t, Kr[h])
            if hh == 0:
                qT[pr] = qkt.tile([P, J, P], F32R, tag=f"qT{pr}", name=f"qT{pr}")
                kT[pr] = qkt.tile([P, J, P], F32R, tag=f"kT{pr}", name=f"kT{pr}")
            for ri, (raw_t, dstT) in enumerate(((qr_t, qT[pr]), (kr_t, kT[pr]))):
                tmp = tmpp.tile([P, J * D], F32, tag="tmp")
                # phi(x) = elu(x) + 1 = max(x + 1, exp(min(x, 0)))
                nc.vector.tensor_scalar_min(tmp[:], raw_t[:], 0.0)
                nc.scalar.activation(tmp[:], tmp[:], AF.Exp)
                nc.vector.scalar_tensor_tensor(
                    raw_t[:], raw_t[:], 1.0, tmp[:], ALU.add, ALU.max
                )
                # transpose 2 j-slots at a time: [128, (2t,2t+1)x64] ->
                # psum rows 0:64 = j=2t d-rows, 64:128 = j=2t+1 d-rows
                for t in range(J // 2):
                    ps = bigp.tile([P, P], F32, tag="big", name="tps")
                    nc.tensor.transpose(
                        ps[:], raw_t[:, 2 * t * D:(2 * t + 2) * D], ident[:]
                    )
                    if (t + ri) % 2 == 0:
                        nc.scalar.copy(dstT[base:base + D, 2 * t, :], ps[0:D, :])
                        nc.vector.tensor_copy(
                            dstT[base:base + D, 2 * t + 1, :], ps[D:2 * D, :]
                        )
                    else:
                        nc.vector.tensor_copy(dstT[base:base + D, 2 * t, :], ps[0:D, :])
                        nc.scalar.copy(dstT[base:base + D, 2 * t + 1, :], ps[D:2 * D, :])
            # bf16 copy of phi(K) row-blocks: lhsT for the bf16 KtV matmuls
            kphi = kphp.tile([P, J * D], BF16, tag="kphi", name=f"kphi{h}")
            nc.vector.tensor_copy(kphi[:], kr_t[:])
            kphis[h] = kphi

        # Emission order chooses per-engine instruction order (each engine
        # runs its stream in-order).  Prep work for head h+2 is interleaved
        # into head h's region so ACT/PE/PSUM aren't hogged by upfront prep.
        prep_head(0)
        prep_head(1)
        load_v(0)
        load_v(1)

        for h in range(H):
            pr, hh = divmod(h, 2)
            base = hh * D  # partition base for this head's d-rows
            kphi = kphis[h]
            v_t = vt[h]

            # KtV[d, v] = sum_n phiK[n, d] V[n, v], accumulated over j-slots
            kv_ps = bigp.tile([P, N], F32, tag="big", name=f"kv{h}")
            for half in range(2):
                for j in range(J):
                    nc.tensor.matmul(
                        kv_ps[base:base + D, half * 512:(half + 1) * 512],
                        kphi[:, j * D:(j + 1) * D],
                        v_t[:, j * N + half * 512:j * N + (half + 1) * 512],
                        start=(j == 0), stop=(j == J - 1),
                    )
            ktv = ktvp.tile([P, N], F32R, tag="ktv", name=f"ktv{h}")
            nc.vector.tensor_copy(ktv[base:base + D, :], kv_ps[base:base + D, :])

            if h + 2 < H:
                load_v(h + 2)
                prep_head(h + 2)

            qTf = qT[pr]
            kTf = kT[pr].rearrange("p j q -> p (j q)")
            out_t = None
            for j in range(J):
                # S chunk: rows n = 8p+j, columns m in (j', p') order
                s_ps = bigp.tile([P, N], F32, tag="big", name=f"sps{h}_{j}")
                for half in range(2):
                    nc.tensor.matmul(
                        s_ps[:, half * 512:(half + 1) * 512],
                        qTf[base:base + D, j, :],
                        kTf[base:base + D, half * 512:(half + 1) * 512],
                        start=True, stop=True,
                    )
                rec = recp.tile([P, N], F32, tag="rec")
                if j in (3, 7):
                    # offload 1/4 of the reciprocals to DVE to balance ACT
                    nc.vector.reciprocal_approx_fast(rec[:], s_ps[:])
                else:
                    lnt = recp.tile([P, N], F32, tag="ln", bufs=1)
                    nc.scalar.activation(lnt[:], s_ps[:], AF.Ln)
                    nc.scalar.activation(rec[:], lnt[:], AF.Exp, scale=-1.0)
                # numerator chunk: natural v order
                n_ps = nps.tile([P, N], F32, tag="nps", name=f"nps{h}_{j}")
                for half in range(2):
                    nc.tensor.matmul(
                        n_ps[:, half * 512:(half + 1) * 512],
                        qTf[base:base + D, j, :],
                        ktv[base:base + D, half * 512:(half + 1) * 512],
                        start=True, stop=True,
                    )
                if j % 2 == 0:
                    out_t = outp.tile([P, 2, N], F32, tag="out", name=f"out{h}_{j // 2}")
                # out[n, v] = num[n, v] * rec[n, m=v]; v = 8*pp + jj maps to
                # rec column (j'=jj, p'=pp) i.e. free index jj*128 + pp
                nc.vector.tensor_mul(
                    out_t[:, j % 2, :].rearrange("p (pp jj) -> p pp jj", jj=J),
                    n_ps.rearrange("p (pp jj) -> p pp jj", jj=J),
                    rec.rearrange("p (jp pp) -> p pp jp", pp=P),
                )
                if j % 2 == 1:
                    nc.sync.dma_start(
                        Orr[h, :, j // 2, :],
                        out_t.rearrange("p a b -> p (a b)"),
                    )
    nc.compile()
    return nc


def _get_nc():
    if "nc" not in _cache:
        _cache["nc"] = _build()
    return _cache["nc"]


def kernel(Q, K, V, _want_trace=False):
    """Takes full inputs Q,K [8,8,1024,64], V [8,8,1024,1024]; returns [8,8,1024,1024]."""
    nc = _get_nc()
    Q = np.ascontiguousarray(np.asarray(Q), dtype=np.float32)
    K = np.ascontiguousarray(np.asarray(K), dtype=np.float32)
    V = np.ascontiguousarray(np.asarray(V), dtype=np.float32)
    in_maps = [
        {"q": Q[b], "k": K[b], "v": V[b]} for b in range(N_CORES)
    ]
    try:
        res = run_bass_kernel_spmd(
            nc, in_maps, core_ids=list(range(N_CORES)), trace=_want_trace
        )
    except ModuleNotFoundError:
        # NTFF profiling hook unavailable in this container; rerun untraced.
        res = run_bass_kernel_spmd(
            nc, in_maps, core_ids=list(range(N_CORES)), trace=False
        )
    out = np.stack([res.results[b]["o"] for b in range(N_CORES)], axis=0)
    if _want_trace:
        _cache["last_result"] = res
    return out


# revision 16
# speedup vs baseline: 1.4949x; 1.0570x over previous
"""Kernel attention (linear attention w/ elu+1 feature map) on 8 trn2 NeuronCores.

Problem: B=8, H=8, N=1024, D=64.
  phi(x) = elu(x) + 1
  S   = phi(Q) @ phi(K)^T          [B,H,N,N]
  out = (S @ V) / S                elementwise divide (dv == N)

Key algebraic rewrite: S has rank D=64, so the numerator is computed as
  numerator = phi(Q) @ (phi(K)^T @ V)
which is ~8x less PE work than materializing S @ V densely.  The full S is
still materialized (in PSUM, chunk by chunk) for the elementwise division,
computed as exp(-ln(S)) on the ACT engine (S > 0 always).

Sharding: batch b -> core b (8 heads per core, fully independent).

Row-block layout: partition p holds rows 8p..8p+7 (j = 0..7), so every
HBM<->SBUF transfer is >=2KB contiguous per partition (full DMA rate):
  - V[h] loads as one 4MB DMA (32KB contiguous per partition)
  - Q[h]/K[h] load as one 256KB DMA each (2KB contiguous per partition)
  - outputs store as [128, 2048] tiles (8KB contiguous per partition)
Per-core dataflow (per head):
  - phiK row-blocks are directly the lhsT for KtV = phi(K)^T @ V (contract
    over n on partitions, accumulated over the 8 j-slots)
  - phiQ/phiK are PE-transposed into qT/kT pair tiles [64(d), j, p] (even
    head on partitions 0:64, odd head on 64:128)
  - per j-chunk (rows n = 8p+j): S = qT-slice^T @ kT (f32r), rec =
    exp(-ln(S)) on ACT, num = qT-slice^T @ KtV (f32r), out = num * rec on
    DVE (with a free-dim permuted view pairing S's (j',p') column order
    with num's natural v order), DMA out per j-pair.
All matmuls run in f32r: full PE rate at free-size >= 256, fp32 storage.
"""

import numpy as np
from contextlib import ExitStack

import concourse.bass as bass
import concourse.tile as tile
import concourse.mybir as mybir
from concourse import bacc
from concourse.bass_utils import run_bass_kernel_spmd
from concourse.masks import make_identity

P = 128
N_CORES = 8
H = 8            # heads per core (batch is the sharded dim)
N = 1024
D = 64
J = N // P       # 8 rows per partition (row-block layout)
F32 = mybir.dt.float32
F32R = mybir.dt.float32r
BF16 = mybir.dt.bfloat16
AF = mybir.ActivationFunctionType
ALU = mybir.AluOpType

_cache = {}


def _patch_act_tables():
    """Force Exp and Ln to resolve to the single table set containing both
    (natural_log_exp_and_others), so the ACT LUT is loaded once instead of
    thrashing ~1.3-2.7us per Ln<->Exp alternation.  Keys/order preserved, so
    act_func_set_id indices stay valid."""
    if _cache.get("tables_patched"):
        return
    orig = bacc.get_activation_tables

    def patched(arch):
        tabs = dict(orig(arch))
        both = [k for k, v in tabs.items() if AF.Exp in v and AF.Ln in v]
        if both:
            keep = both[0]
            tabs = {
                k: (v if k == keep else (set(v) - {AF.Exp, AF.Ln}))
                for k, v in tabs.items()
            }
        return tabs

    bacc.get_activation_tables = patched
    _cache["tables_patched"] = True


def _build():
    _patch_act_tables()
    nc = bacc.Bacc("TRN2", target_bir_lowering=False, debug=False, num_devices=N_CORES)
    Q = nc.dram_tensor("q", [H, N, D], F32, kind="ExternalInput").ap()
    K = nc.dram_tensor("k", [H, N, D], F32, kind="ExternalInput").ap()
    V = nc.dram_tensor("v", [H, N, N], F32, kind="ExternalInput").ap()
    O = nc.dram_tensor("o", [H, N, N], F32, kind="ExternalOutput").ap()

    # Row-block views: partition p <- rows 8p..8p+7.
    Qr = Q.rearrange("h (p j) d -> h p (j d)", p=P)                  # [8, 128, 512]
    Kr = K.rearrange("h (p j) d -> h p (j d)", p=P)
    Vr = V.rearrange("h (p j) v -> h p (j v)", p=P)                  # [8, 128, 8192]
    Orr = O.rearrange("h (p jj jt) v -> h p jj (jt v)", p=P, jt=2)   # [8, 128, 4, 2048]

    with tile.TileContext(nc) as tc, ExitStack() as ctx:
        const = ctx.enter_context(tc.tile_pool(name="const", bufs=1))
        rawq = ctx.enter_context(tc.tile_pool(name="rawq", bufs=2))
        rawk = ctx.enter_context(tc.tile_pool(name="rawk", bufs=2))
        # kphi (bf16 phi(K), lhsT of KtV) persists until its head's KtV
        kphp = ctx.enter_context(tc.tile_pool(name="kphp", bufs=8))
        tmpp = ctx.enter_context(tc.tile_pool(name="tmpp", bufs=2))
        qkt = ctx.enter_context(tc.tile_pool(name="qkt", bufs=1))
        ktvp = ctx.enter_context(tc.tile_pool(name="ktvp", bufs=2))
        vp = ctx.enter_context(tc.tile_pool(name="vp", bufs=2))
        recp = ctx.enter_context(tc.tile_pool(name="recp", bufs=2))
        outp = ctx.enter_context(tc.tile_pool(name="outp", bufs=2))
        # PSUM: "big" ring (4 banks) shared by transposes / KtV / S chunks,
        # "nps" ring (4 banks) for numerator chunks.
        bigp = ctx.enter_context(tc.tile_pool(name="bigp", bufs=2, space="PSUM"))
        nps = ctx.enter_context(tc.tile_pool(name="nps", bufs=2, space="PSUM"))

        ident = const.tile([P, P], F32)
        make_identity(nc, ident)

        vt = [None] * H

        def load_v(h):
            # SWDGE load with fp32->bf16 cast (halves SBUF-side DMA bytes)
            v_t = vp.tile([P, J * N], BF16, tag="vt", name=f"vt{h}")
            nc.gpsimd.dma_start(v_t, Vr[h])
            vt[h] = v_t

        # Pair tiles: even head's 64 d-rows on partitions 0:64, odd on 64:128.
        qT = [None] * (H // 2)
        kT = [None] * (H // 2)
        kphis = [None] * H
        raws = [None] * H

        def prep_load(h):
            """Issue the Q/K DMA loads for head h."""
            qr_t = rawq.tile([P, J * D], F32, tag="qraw", name=f"qraw{h}")
            kr_t = rawk.tile([P, J * D], F32, tag="kraw", name=f"kraw{h}")
            nc.sync.dma_start(qr_t, Qr[h])
            nc.sync.dma_start(kr_t, Kr[h])
            raws[h] = (qr_t, kr_t)

        def prep_compute(h):
            """Apply phi to head h's Q/K and build its half of qT/kT."""
            pr, hh = divmod(h, 2)
            base = hh * D
            qr_t, kr_t = raws[h]
            if hh == 0:
                qT[pr] = qkt.tile([P, J, P], F32R, tag=f"qT{pr}", name=f"qT{pr}")
                kT[pr] = qkt.tile([P, J, P], F32R, tag=f"kT{pr}", name=f"kT{pr}")
            for ri, (raw_t, dstT) in enumerate(((qr_t, qT[pr]), (kr_t, kT[pr]))):
                tmp = tmpp.tile([P, J * D], F32, tag="tmp")
                # phi(x) = elu(x) + 1 = max(x + 1, exp(min(x, 0)))
                nc.vector.tensor_scalar_min(tmp[:], raw_t[:], 0.0)
                nc.scalar.activation(tmp[:], tmp[:], AF.Exp)
                nc.vector.scalar_tensor_tensor(
                    raw_t[:], raw_t[:], 1.0, tmp[:], ALU.add, ALU.max
                )
                # transpose 2 j-slots at a time: [128, (2t,2t+1)x64] ->
                # psum rows 0:64 = j=2t d-rows, 64:128 = j=2t+1 d-rows
                for t in range(J // 2):
                    ps = bigp.tile([P, P], F32, tag="big", name="tps")
                    nc.tensor.transpose(
                        ps[:], raw_t[:, 2 * t * D:(2 * t + 2) * D], ident[:]
                    )
                    if (t + ri) % 2 == 0:
                        nc.scalar.copy(dstT[base:base + D, 2 * t, :], ps[0:D, :])
                        nc.vector.tensor_copy(
                            dstT[base:base + D, 2 * t + 1, :], ps[D:2 * D, :]
                        )
                    else:
                        nc.vector.tensor_copy(dstT[base:base + D, 2 * t, :], ps[0:D, :])
                        nc.scalar.copy(dstT[base:base + D, 2 * t + 1, :], ps[D:2 * D, :])
            # bf16 copy of phi(K) row-blocks: lhsT for the bf16 KtV matmuls
            kphi = kphp.tile([P, J * D], BF16, tag="kphi", name=f"kphi{h}")
            nc.vector.tensor_copy(kphi[:], kr_t[:])
            kphis[h] = kphi

        # Emission order chooses per-engine instruction order (each engine
        # runs its stream in-order).  Prep loads for head h+2 are issued
        # early in head h's region; prep compute runs after head h's j-loop
        # so it never blocks the ACT Ln/Exp stream of the current head.
        prep_load(0)
        prep_load(1)
        prep_compute(0)
        prep_compute(1)
        load_v(0)
        load_v(1)

        for h in range(H):
            pr, hh = divmod(h, 2)
            base = hh * D  # partition base for this head's d-rows
            kphi = kphis[h]
            v_t = vt[h]

            # KtV[d, v] = sum_n phiK[n, d] V[n, v], accumulated over j-slots
            kv_ps = bigp.tile([P, N], F32, tag="big", name=f"kv{h}")
            for half in range(2):
                for j in range(J):
                    nc.tensor.matmul(
                        kv_ps[base:base + D, half * 512:(half + 1) * 512],
                        kphi[:, j * D:(j + 1) * D],
                        v_t[:, j * N + half * 512:j * N + (half + 1) * 512],
                        start=(j == 0), stop=(j == J - 1),
                    )
            ktv = ktvp.tile([P, N], F32R, tag="ktv", name=f"ktv{h}")
            nc.vector.tensor_copy(ktv[base:base + D, :], kv_ps[base:base + D, :])

            if h + 2 < H:
                load_v(h + 2)
                prep_load(h + 2)

            qTf = qT[pr]
            kTf = kT[pr].rearrange("p j q -> p (j q)")
            out_t = None
            for j in range(J):
                # S chunk: rows n = 8p+j, columns m in (j', p') order
                s_ps = bigp.tile([P, N], F32, tag="big", name=f"sps{h}_{j}")
                for half in range(2):
                    nc.tensor.matmul(
                        s_ps[:, half * 512:(half + 1) * 512],
                        qTf[base:base + D, j, :],
                        kTf[base:base + D, half * 512:(half + 1) * 512],
                        start=True, stop=True,
                    )
                rec = recp.tile([P, N], F32, tag="rec")
                if j in (3, 7):
                    # offload 1/4 of the reciprocals to DVE to balance ACT
                    nc.vector.reciprocal_approx_fast(rec[:], s_ps[:])
                else:
                    lnt = recp.tile([P, N], F32, tag="ln", bufs=1)
                    nc.scalar.activation(lnt[:], s_ps[:], AF.Ln)
                    nc.scalar.activation(rec[:], lnt[:], AF.Exp, scale=-1.0)
                # numerator chunk: natural v order
                n_ps = nps.tile([P, N], F32, tag="nps", name=f"nps{h}_{j}")
                for half in range(2):
                    nc.tensor.matmul(
                        n_ps[:, half * 512:(half + 1) * 512],
                        qTf[base:base + D, j, :],
                        ktv[base:base + D, half * 512:(half + 1) * 512],
                        start=True, stop=True,
                    )
                if j % 2 == 0:
                    out_t = outp.tile([P, 2, N], F32, tag="out", name=f"out{h}_{j // 2}")
                # out[n, v] = num[n, v] * rec[n, m=v]; v = 8*pp + jj maps to
                # rec column (j'=jj, p'=pp) i.e. free index jj*128 + pp
                nc.vector.tensor_mul(
                    out_t[:, j % 2, :].rearrange("p (pp jj) -> p pp jj", jj=J),
                    n_ps.rearrange("p (pp jj) -> p pp jj", jj=J),
                    rec.rearrange("p (jp pp) -> p pp jp", pp=P),
                )
                if j % 2 == 1:
                    nc.sync.dma_start(
                        Orr[h, :, j // 2, :],
                        out_t.rearrange("p a b -> p (a b)"),
                    )
            if h + 2 < H:
                prep_compute(h + 2)
    nc.compile()
    return nc


def _get_nc():
    if "nc" not in _cache:
        _cache["nc"] = _build()
    return _cache["nc"]


def kernel(Q, K, V, _want_trace=False):
    """Takes full inputs Q,K [8,8,1024,64], V [8,8,1024,1024]; returns [8,8,1024,1024]."""
    nc = _get_nc()
    Q = np.ascontiguousarray(np.asarray(Q), dtype=np.float32)
    K = np.ascontiguousarray(np.asarray(K), dtype=np.float32)
    V = np.ascontiguousarray(np.asarray(V), dtype=np.float32)
    in_maps = [
        {"q": Q[b], "k": K[b], "v": V[b]} for b in range(N_CORES)
    ]
    try:
        res = run_bass_kernel_spmd(
            nc, in_maps, core_ids=list(range(N_CORES)), trace=_want_trace
        )
    except ModuleNotFoundError:
        # NTFF profiling hook unavailable in this container; rerun untraced.
        res = run_bass_kernel_spmd(
            nc, in_maps, core_ids=list(range(N_CORES)), trace=False
        )
    out = np.stack([res.results[b]["o"] for b in range(N_CORES)], axis=0)
    if _want_trace:
        _cache["last_result"] = res
    return out
